# revision 29
# baseline (speedup 1.0000x reference)
"""Multi-head attention (B=2, N=2048, C=1024, H=16, D=64) on 8 TRN2 NeuronCores.

Sharding: core = b*4 + g  (b in {0,1} data-parallel over batch,
g in {0..3} tensor-parallel over head groups of HL=4 heads).

Per-core computation (all layouts chosen so the matmul contraction dim is
always on SBUF partitions, with no on-chip transposes; matmuls run in
float32r = TF32-like full-rate mode):
  phase 0: QT = wqT.T @ xT  -> [DL, N] (d on partitions; bias + 1/sqrt(D)
           folded in), KT likewise, V = xT.T @ wvT -> [N, DL] + ones col
  per head h, per m-chunk i (128 tokens):
    A: S^T = KT_h[:, i].T @ QT_h -> psum [128, 1024] x2 halves
       E^T = exp(S^T) via ScalarE -> short-lived SBUF tile e_i [128, 2048]
    B: O_aug^T[nb] += V_aug[i].T @ e_i[:, nb]  (4 psum accumulators [65,512]:
       rows 0..63 = O^T, row 64 = softmax denominator)
  norm: Y^T = O^T * (1/denom); reciprocal_approx_fast + K=1 ones-matmul
        partition-broadcast
  phase D: P^T = pwT.T @ Y^T -> partial projection [C, N]

Host: out[b] = sum_g P^T[b,g].T + proj_b
"""

import numpy as np
import ml_dtypes

B, N, C = 2, 2048, 1024
H = 16
D = C // H          # 64
G = 4               # head groups (tensor parallel)
HL = H // G         # 4 heads per core
DL = HL * D         # 256 local head dims
N_CORES = 8
SCALE = 1.0 / np.sqrt(np.float32(D))

MCHUNKS = N // 128  # 16

_CACHE = {}
DEBUG_TAPS = False


def build_kernel():
    import concourse.bass as bass
    import concourse.mybir as mybir
    import concourse.tile as tile
    from concourse import bacc

    f32 = mybir.dt.float32
    f32r = mybir.dt.float32r
    bf16 = mybir.dt.bfloat16

    nc = bacc.Bacc("TRN2", target_bir_lowering=False, debug=False,
                   num_devices=N_CORES)

    xt_d = nc.dram_tensor("xt", [C, N], bf16, kind="ExternalInput").ap()
    wqt_d = nc.dram_tensor("wqt", [C, DL], bf16, kind="ExternalInput").ap()
    wkt_d = nc.dram_tensor("wkt", [C, DL], bf16, kind="ExternalInput").ap()
    wvt_d = nc.dram_tensor("wvt", [C, DL], bf16, kind="ExternalInput").ap()
    bq_d = nc.dram_tensor("bq", [128, DL // 128], f32, kind="ExternalInput").ap()
    bk_d = nc.dram_tensor("bk", [128, DL // 128], f32, kind="ExternalInput").ap()
    bv_d = nc.dram_tensor("bv", [1, DL], f32r, kind="ExternalInput").ap()
    pwt_d = nc.dram_tensor("pwt", [DL, C], f32r, kind="ExternalInput").ap()
    out_d = nc.dram_tensor("out", [C, N], f32, kind="ExternalOutput").ap()
    if DEBUG_TAPS:
        dbg = {
            "dbg_qt": nc.dram_tensor("dbg_qt", [128, DL // 128, N], f32r,
                                     kind="ExternalOutput").ap(),
            "dbg_kt": nc.dram_tensor("dbg_kt", [128, DL // 128, N], f32r,
                                     kind="ExternalOutput").ap(),
            "dbg_v": nc.dram_tensor("dbg_v", [128, MCHUNKS, HL, D + 1], f32r,
                                    kind="ExternalOutput").ap(),
            "dbg_e0": nc.dram_tensor("dbg_e0", [128, N], f32r,
                                     kind="ExternalOutput").ap(),
            "dbg_ob": nc.dram_tensor("dbg_ob", [D + 1, 4, 512], f32,
                                     kind="ExternalOutput").ap(),
            "dbg_yt": nc.dram_tensor("dbg_yt", [128, DL // 128, N], f32r,
                                     kind="ExternalOutput").ap(),
            "dbg_rc": nc.dram_tensor("dbg_rc", [1, 4, 512], f32,
                                     kind="ExternalOutput").ap(),
            "dbg_bc": nc.dram_tensor("dbg_bc", [64, 4, 512], f32,
                                     kind="ExternalOutput").ap(),
        }

    CO = C // 128   # 8 chunks of the contraction dim c
    MO = DL // 128  # 2 chunks of the local head dims

    with tile.TileContext(nc) as tc:
        with (
            tc.tile_pool(name="consts", bufs=1) as consts,
            tc.tile_pool(name="acts", bufs=1) as acts,
            tc.tile_pool(name="small", bufs=4) as small,
            tc.tile_pool(name="stage", bufs=3) as stage,
            tc.tile_pool(name="psS", bufs=2, space="PSUM") as psS,
            tc.tile_pool(name="psB", bufs=4, space="PSUM") as psB,
        ):
            # ---- load weights/bias constants ----
            wq_sb = consts.tile([128, CO, DL], bf16, tag="wq")
            wk_sb = consts.tile([128, CO, DL], bf16, tag="wk")
            wv_sb = consts.tile([128, CO, DL], bf16, tag="wv")
            nc.sync.dma_start(wq_sb[:], wqt_d.rearrange("(o p) f -> p o f", p=128))
            nc.sync.dma_start(wk_sb[:], wkt_d.rearrange("(o p) f -> p o f", p=128))
            nc.sync.dma_start(wv_sb[:], wvt_d.rearrange("(o p) f -> p o f", p=128))
            pw_sb = consts.tile([128, MO, C], f32r, tag="pw")
            nc.sync.dma_start(pw_sb[:], pwt_d.rearrange("(o p) f -> p o f", p=128))
            bq_sb = consts.tile([128, MO], f32, tag="bq")
            bk_sb = consts.tile([128, MO], f32, tag="bk")
            nc.sync.dma_start(bq_sb[:], bq_d[:])
            nc.sync.dma_start(bk_sb[:], bk_d[:])
            bv_sb = consts.tile([1, DL], f32r, tag="bv")
            nc.sync.dma_start(bv_sb[:], bv_d[:])
            ones_f = consts.tile([1, 128], f32, tag="onesf")
            nc.vector.memset(ones_f[:], 1.0)
            ones_sb = consts.tile([1, 128], f32r, tag="ones")
            nc.vector.tensor_copy(ones_sb[:], ones_f[:])

            # ---- activations that stay resident ----
            qt_sb = acts.tile([128, MO, N], f32r, tag="qt")   # [DL, N]
            kt_sb = acts.tile([128, MO, N], f32r, tag="kt")   # [DL, N]
            v_sb = acts.tile([128, MCHUNKS, HL, D + 1], f32r, tag="v")
            yt_sb = acts.tile([128, MO, N], f32r, tag="yt")   # [DL, N] normalized

            ones_col = consts.tile([128, 1], f32, tag="onescol")
            nc.vector.memset(ones_col[:], 1.0)
            nc.vector.tensor_copy(
                v_sb[:, :, :, D:],
                ones_col[:].to_broadcast([128, MCHUNKS, HL, 1]))

            # ---- phase 0 ----
            with tc.tile_pool(name="xt", bufs=1) as xt_pool:
                xt_sb = xt_pool.tile([128, CO, N], bf16, tag="xt")
                xt_r = xt_d.rearrange("(o p) n -> p o n", p=128)
                for kc in range(CO):
                    nc.sync.dma_start(xt_sb[:, kc, :], xt_r[:, kc, :])

                for w_sb, b_sb, o_sb in ((wq_sb, bq_sb, qt_sb),
                                         (wk_sb, bk_sb, kt_sb)):
                    for mo in range(MO):
                        for nh in range(2):
                            ps = psS.tile([128, 1024], f32, tag="pss")
                            for kc in range(CO):
                                for half in range(2):
                                    nc.tensor.matmul(
                                        ps[:, half * 512:(half + 1) * 512],
                                        lhsT=w_sb[:, kc, mo * 128:(mo + 1) * 128],
                                        rhs=xt_sb[:, kc,
                                                  nh * 1024 + half * 512:
                                                  nh * 1024 + (half + 1) * 512],
                                        start=(kc == 0), stop=(kc == CO - 1),
                                    )
                            nc.vector.tensor_scalar_add(
                                o_sb[:, mo, nh * 1024:(nh + 1) * 1024], ps[:],
                                b_sb[:, mo:mo + 1],
                            )
                # V (token m on partitions) + ones-matmul bias broadcast
                for i in range(MCHUNKS):
                    ps = psS.tile([128, HL, D], f32, tag="pss")
                    for kc in range(CO):
                        nc.tensor.matmul(
                            ps[:],
                            lhsT=xt_sb[:, kc, i * 128:(i + 1) * 128],
                            rhs=wv_sb[:, kc, :],
                            start=(kc == 0), stop=False,
                        )
                    nc.tensor.matmul(
                        ps[:], lhsT=ones_sb[:], rhs=bv_sb[:],
                        start=False, stop=True,
                    )
                    nc.vector.tensor_copy(v_sb[:, i, :, :D], ps[:])

            if DEBUG_TAPS:
                nc.sync.dma_start(dbg["dbg_qt"][:], qt_sb[:])
                nc.sync.dma_start(dbg["dbg_kt"][:], kt_sb[:])
                nc.sync.dma_start(dbg["dbg_v"][:], v_sb[:])

            # ---- attention: per head, m-chunk-streamed; norm(h) is emitted
            # inside head h+1's stream so the PE never idles long enough to
            # trip the HAM clock-gate back to 1.2 GHz ----
            def emit_D_block(nb):
                nsl = slice(nb * 512, (nb + 1) * 512)
                for cc in range(8):
                    ps = psS.tile([128, 1024], f32, tag="pss",
                                  name=f"psd_{nb}_{cc}")
                    for jc in range(MO):
                        nc.tensor.matmul(
                            ps[:, :512],
                            lhsT=pw_sb[:, jc, cc * 128:(cc + 1) * 128],
                            rhs=yt_sb[:, jc, nsl],
                            start=(jc == 0), stop=(jc == MO - 1),
                        )
                    st = stage.tile([128, 512], f32, tag="st")
                    if cc % 2 == 0:
                        nc.scalar.activation(
                            st[:], ps[:, :512],
                            mybir.ActivationFunctionType.Copy)
                    else:
                        nc.vector.tensor_copy(st[:], ps[:, :512])
                    nc.sync.dma_start(
                        out_d[cc * 128:(cc + 1) * 128, nsl], st[:])

            def emit_norm(hn, psBs_n, chain_d=False):
                mo_n = hn // 2
                pb_n = 64 * (hn % 2)
                if DEBUG_TAPS and hn == 0:
                    for nb in range(4):
                        obf = small.tile([D + 1, 512], f32, tag="dbgob",
                                         name=f"obf{nb}")
                        nc.vector.tensor_copy(obf[:], psBs_n[nb][:])
                        nc.sync.dma_start(dbg["dbg_ob"][:, nb, :], obf[:])
                for nb in range(4):
                    nsl = slice(nb * 512, (nb + 1) * 512)
                    dn = small.tile([1, 512], f32, tag="dn")
                    nc.vector.tensor_copy(dn[:], psBs_n[nb][D:D + 1, :])
                    rc = small.tile([1, 512], f32, tag="rc")
                    nc.vector.reciprocal_approx_fast(rc[:], dn[:])
                    if DEBUG_TAPS and hn == 0:
                        nc.sync.dma_start(dbg["dbg_rc"][:, nb, :], rc[:])
                    ot = small.tile([64, 512], f32, tag="ot")
                    nc.vector.tensor_copy(ot[:], psBs_n[nb][:D, :])
                    bc = psB.tile([64, 512], f32, tag="psb",
                                  name=f"bc_{hn}_{nb}")
                    nc.tensor.matmul(bc[:], lhsT=ones_f[:, :64],
                                     rhs=rc[:], start=True, stop=True)
                    if DEBUG_TAPS and hn == 0:
                        bcf = small.tile([64, 512], f32, tag="dbgob",
                                         name=f"bcf{nb}")
                        nc.vector.tensor_copy(bcf[:], bc[:])
                        nc.sync.dma_start(dbg["dbg_bc"][:, nb, :], bcf[:])
                    nc.vector.tensor_mul(
                        yt_sb[pb_n:pb_n + D, mo_n, nsl], ot[:], bc[:])
                    if chain_d:
                        emit_D_block(nb)

            with (
                tc.tile_pool(name="ei", bufs=3) as ei_pool,
            ):
                # software pipeline: phase B for m-chunk i-1 is emitted
                # alongside phase A for chunk i, so the PE never stalls on
                # the exp results it just requested.
                psBs_by_h = {}
                pending = None   # (h, i, ei) awaiting its B matmuls

                def emit_B(hb, ib, eib):
                    if ib == 0:
                        if hb > 0:
                            emit_norm(hb - 1, psBs_by_h.pop(hb - 1))
                        psBs_by_h[hb] = [
                            psB.tile([D + 1, 512], f32, tag="psb",
                                     name=f"psb_{hb}_{nb}")
                            for nb in range(4)]
                    for nb in range(4):
                        nc.tensor.matmul(
                            psBs_by_h[hb][nb][:],
                            lhsT=v_sb[:, ib, hb, :],
                            rhs=eib[:, nb * 512:(nb + 1) * 512],
                            start=(ib == 0), stop=(ib == MCHUNKS - 1),
                        )

                for h in range(HL):
                    mo = h // 2
                    pb = 64 * (h % 2)
                    for i in range(MCHUNKS):
                        ei = ei_pool.tile([128, N], f32r, tag="ei")
                        for nh in range(2):
                            ps = psS.tile([128, 1024], f32, tag="pss")
                            for half in range(2):
                                nc.tensor.matmul(
                                    ps[:, half * 512:(half + 1) * 512],
                                    lhsT=kt_sb[pb:pb + D, mo,
                                               i * 128:(i + 1) * 128],
                                    rhs=qt_sb[pb:pb + D, mo,
                                              nh * 1024 + half * 512:
                                              nh * 1024 + (half + 1) * 512],
                                    start=True, stop=True,
                                )
                            nc.scalar.activation(
                                ei[:, nh * 1024:(nh + 1) * 1024], ps[:],
                                mybir.ActivationFunctionType.Exp,
                            )
                        if DEBUG_TAPS and h == 0 and i == 0:
                            nc.sync.dma_start(dbg["dbg_e0"][:], ei[:])
                        if pending is not None:
                            emit_B(*pending)
                        pending = (h, i, ei)
                emit_B(*pending)
                emit_norm(HL - 1, psBs_by_h.pop(HL - 1), chain_d=True)

                if DEBUG_TAPS:
                    nc.sync.dma_start(dbg["dbg_yt"][:], yt_sb[:])

    nc.compile()
    return nc


def shard_inputs(x, qkv_w, qkv_b, proj_w):
    """Build the 8 per-core input maps (host-side sharding)."""
    in_maps = []
    for core in range(N_CORES):
        b, g = divmod(core, G)
        gs = slice(g * DL, (g + 1) * DL)
        xt = np.ascontiguousarray(x[b].T)
        wq = qkv_w[0 * C:1 * C][gs] * SCALE     # fold 1/sqrt(D) into Q
        wk = qkv_w[1 * C:2 * C][gs]
        wv = qkv_w[2 * C:3 * C][gs]
        in_maps.append({
            "xt": np.ascontiguousarray(xt).astype(ml_dtypes.bfloat16),
            "wqt": np.ascontiguousarray(wq.T).astype(ml_dtypes.bfloat16),
            "wkt": np.ascontiguousarray(wk.T).astype(ml_dtypes.bfloat16),
            "wvt": np.ascontiguousarray(wv.T).astype(ml_dtypes.bfloat16),
            "bq": np.ascontiguousarray(
                (qkv_b[0 * C:1 * C][gs] * SCALE).reshape(DL // 128, 128).T),
            "bk": np.ascontiguousarray(
                qkv_b[1 * C:2 * C][gs].reshape(DL // 128, 128).T),
            "bv": np.ascontiguousarray(qkv_b[2 * C:3 * C][gs].reshape(1, DL)),
            "pwt": np.ascontiguousarray(proj_w[:, gs].T),
        })
    return in_maps


def unshard_output(results, proj_b):
    """results: list of 8 dicts with 'out' [C, N] partial projections."""
    out = np.empty((B, N, C), dtype=np.float32)
    for b in range(B):
        acc = results[b * G]["out"].astype(np.float32)
        for g in range(1, G):
            acc = acc + results[b * G + g]["out"]
        out[b] = acc.T + proj_b
    return out


def kernel(x, qkv_w, qkv_b, proj_w, proj_b):
    from concourse.bass_utils import run_bass_kernel_spmd

    x = np.asarray(x, dtype=np.float32)
    qkv_w = np.asarray(qkv_w, dtype=np.float32)
    qkv_b = np.asarray(qkv_b, dtype=np.float32)
    proj_w = np.asarray(proj_w, dtype=np.float32)
    proj_b = np.asarray(proj_b, dtype=np.float32)

    if "nc" not in _CACHE:
        _CACHE["nc"] = build_kernel()
    nc = _CACHE["nc"]

    in_maps = shard_inputs(x, qkv_w, qkv_b, proj_w)
    res = run_bass_kernel_spmd(nc, in_maps, list(range(N_CORES)))
    return unshard_output(res.results, proj_b)
